# revision 20
# baseline (speedup 1.0000x reference)
"""Self-contained Trainium2 Bass kernel for causal MHA.

Problem: B=32, L=512, D=4096, H=32 heads (head_dim 128), causal attention,
torch-Linear projections (y = x @ W.T + b).

Strategy: data-parallel over batch across the 8 NeuronCores (4 batches each).
Per core, everything is computed in "transposed activation" layout so no
on-chip transposes are ever needed:
  - Q.T, K.T per head:  [head_dim(part), tok]   (lhsT = W.T tile, rhs = x.T)
  - V natural per head-group: [tok(part), feat]  (lhsT = x.T tile, rhs = Wv.T)
  - S.T = K.T-chunk.T @ Q.T -> [key(part), query]; softmax denominator via an
    all-ones stationary matmul (broadcasts column sums to all partitions);
    causal handled by a multiplicative mask after exp.
  - out.T = V-chunk.T @ p.T accumulated over key chunks -> [head_dim, tok],
    normalized by the reciprocal of the ones-matmul output.
  - y = attout.T-tile.T @ Wo.T chunk (natural layout), + bias, DMA out.
Matmuls run in bf16 (fp32 accumulate in PSUM).

Perf structure:
  - All DRAM tensors pre-arranged on host so every dma_start is a contiguous
    per-partition transfer (one large descriptor per partition).
  - wv/wo weight tiles are loaded once per (batch, chunk) and streamed against
    4 concurrent PSUM accumulation banks (tag "acc", 4 slots) so no weight is
    fetched twice.
  - The softmax denominator PSUM bank is drained by a fast ScalarE copy before
    the slow VectorE reciprocal, so the bank frees in ~0.7us instead of 3.4us
    (which used to stall the next head's matmuls on PSUM slot reuse).
"""

import os
import sys

sys.path.insert(0, "/opt/trn_rl_repo")

import numpy as np
import ml_dtypes

import concourse.bass as bass
import concourse.mybir as mybir
import concourse.tile as tile
from concourse import bacc
from concourse import bass_utils
from concourse.bass_interp import get_hw_module

BF16 = mybir.dt.bfloat16
F32 = mybir.dt.float32
NPBF16 = ml_dtypes.bfloat16
AFT = mybir.ActivationFunctionType

B, L, D, H = 32, 512, 4096, 32
HD = 128
NCORES = 8
BC = B // NCORES          # batches per core
KO = D // 128             # 32 contraction tiles
NCH = D // 512            # 8 output-feature chunks of 512
NG = H // 4               # 8 head groups of 4 heads
SCALE = 1.0 / float(np.sqrt(HD))

_CACHE = {}


def _build():
    nc = bacc.Bacc(
        "TRN2", target_bir_lowering=False, debug=False, enable_asserts=False
    )
    # Pre-transposed on host: per-partition rows are fully contiguous.
    xT = nc.dram_tensor("xT", [BC, 128, KO, 512], BF16, kind="ExternalInput").ap()
    wq = nc.dram_tensor("wq", [H, 128, KO, 128], BF16, kind="ExternalInput").ap()
    wk = nc.dram_tensor("wk", [H, 128, KO, 128], BF16, kind="ExternalInput").ap()
    wv = nc.dram_tensor("wv", [NCH, 128, KO, 512], BF16, kind="ExternalInput").ap()
    wo = nc.dram_tensor("wo", [NCH, 128, KO, 512], BF16, kind="ExternalInput").ap()
    bqr = nc.dram_tensor("bqr", [128, H], F32, kind="ExternalInput").ap()
    bkr = nc.dram_tensor("bkr", [128, H], F32, kind="ExternalInput").ap()
    bvb = nc.dram_tensor("bvb", [128, D], BF16, kind="ExternalInput").ap()
    bob = nc.dram_tensor("bob", [128, D], BF16, kind="ExternalInput").ap()
    trid = nc.dram_tensor("trid", [128, 128], BF16, kind="ExternalInput").ap()
    onesd = nc.dram_tensor("onesd", [128, 128], BF16, kind="ExternalInput").ap()
    y = nc.dram_tensor("y", [BC * 512, D], BF16, kind="ExternalOutput").ap()

    ts = bass.ts

    with tile.TileContext(nc) as tc:
        with tc.tile_pool(name="const", bufs=1) as constp, \
             tc.tile_pool(name="xpool", bufs=1) as xpool, \
             tc.tile_pool(name="wqk", bufs=2) as wqkp, \
             tc.tile_pool(name="wstream", bufs=3) as wsp, \
             tc.tile_pool(name="qk", bufs=5) as qkp, \
             tc.tile_pool(name="vg", bufs=2) as vgp, \
             tc.tile_pool(name="pt", bufs=20) as ptp, \
             tc.tile_pool(name="rr", bufs=2) as rrp, \
             tc.tile_pool(name="att", bufs=1) as attp, \
             tc.tile_pool(name="yout", bufs=4) as youtp, \
             tc.tile_pool(name="psQ", bufs=2, space="PSUM") as psQ, \
             tc.tile_pool(name="psG", bufs=4, space="PSUM") as psG:

            consts_loaded = False

            for b in range(BC):
                xT_sb = xpool.tile([128, KO, 512], BF16, tag="xT")
                for kb in range(4):
                    nc.sync.dma_start(
                        xT_sb[:, 8 * kb:8 * kb + 8, :],
                        xT[b, :, 8 * kb:8 * kb + 8, :],
                    )
                attout = attp.tile([128, H, 512], BF16, tag="attout")

                for g in range(NG):
                    # ---- V for the 4 heads of this group: [tok, 512 feats].
                    # One weight fetch per kb chunk feeds all 4 token tiles.
                    v_sb = vgp.tile([128, 4, 512], BF16, tag="vg")
                    v_ps = [psG.tile([128, 512], F32, tag="acc",
                                     name=f"v_ps{t}")
                            for t in range(4)]
                    for kb in range(KO // 8):
                        wv_t = wsp.tile([128, 8, 512], BF16, tag="wst")
                        nc.sync.dma_start(
                            wv_t[:], wv[g, :, 8 * kb:8 * kb + 8, :]
                        )
                        if not consts_loaded:
                            # issued after the first compute-critical loads so
                            # the first V matmuls start as early as possible
                            consts_loaded = True
                            tri_sb = constp.tile([128, 128], BF16)
                            nc.sync.dma_start(tri_sb[:], trid[:])
                            ones_sb = constp.tile([128, 128], BF16)
                            nc.sync.dma_start(ones_sb[:], onesd[:])
                            bq_sb = constp.tile([128, H], F32)
                            nc.sync.dma_start(bq_sb[:], bqr[:])
                            bk_sb = constp.tile([128, H], F32)
                            nc.sync.dma_start(bk_sb[:], bkr[:])
                            bv_sb = constp.tile([128, D], BF16)
                            nc.sync.dma_start(bv_sb[:], bvb[:])
                            bo_sb = constp.tile([128, D], BF16)
                            nc.sync.dma_start(bo_sb[:], bob[:])
                        for r in range(8):
                            ko = 8 * kb + r
                            for t in range(4):
                                nc.tensor.matmul(
                                    v_ps[t][:], xT_sb[:, ko, ts(t, 128)],
                                    wv_t[:, r, :],
                                    start=(ko == 0), stop=(ko == KO - 1),
                                )
                    for t in range(4):
                        nc.vector.tensor_add(
                            v_sb[:, t, :], v_ps[t][:], bv_sb[:, ts(g, 512)]
                        )

                    # ---- Q.T/K.T for all 4 heads first (gives PE runway to
                    # hide the per-head softmax vector work below)
                    qs, ks = [], []
                    for hh in range(4):
                        h = 4 * g + hh
                        wq_sb = wqkp.tile([128, KO, 128], BF16, tag="w")
                        nc.sync.dma_start(wq_sb[:], wq[h])
                        q_ps = psQ.tile([128, 512], F32, tag="qk")
                        for ko in range(KO):
                            nc.tensor.matmul(
                                q_ps[:], wq_sb[:, ko, :], xT_sb[:, ko, :],
                                start=(ko == 0), stop=(ko == KO - 1),
                            )
                        q_sb = qkp.tile([128, 512], BF16, tag="q")
                        nc.scalar.activation(
                            q_sb[:], q_ps[:], AFT.Identity,
                            bias=bq_sb[:, h:h + 1],
                        )
                        qs.append(q_sb)

                        wk_sb = wqkp.tile([128, KO, 128], BF16, tag="w")
                        nc.sync.dma_start(wk_sb[:], wk[h])
                        k_ps = psQ.tile([128, 512], F32, tag="qk")
                        for ko in range(KO):
                            nc.tensor.matmul(
                                k_ps[:], wk_sb[:, ko, :], xT_sb[:, ko, :],
                                start=(ko == 0), stop=(ko == KO - 1),
                            )
                        k_sb = qkp.tile([128, 512], BF16, tag="k")
                        nc.scalar.activation(
                            k_sb[:], k_ps[:], AFT.Identity,
                            bias=bk_sb[:, h:h + 1],
                        )
                        ks.append(k_sb)

                    # ---- S.T + exp + causal mask for ALL 4 heads first, so
                    # the R/av matmuls of head hh overlap the exp/mask vector
                    # chains of heads hh+1..  Chunk c only contributes to
                    # queries >= 128c (causal): N_c = 512-128c columns.
                    ptss = []
                    for hh in range(4):
                        q_sb, k_sb = qs[hh], ks[hh]
                        pts = []
                        for c in range(4):
                            n_c = 512 - 128 * c
                            st_ps = psQ.tile([128, 512], F32, tag="st")
                            nc.tensor.matmul(
                                st_ps[:, 0:n_c], k_sb[:, ts(c, 128)],
                                q_sb[:, 128 * c:512],
                                start=True, stop=True,
                            )
                            pt_c = ptp.tile([128, 512], BF16, tag="pt")
                            nc.scalar.activation(
                                pt_c[:, 0:n_c], st_ps[:, 0:n_c],
                                AFT.Exp, scale=SCALE,
                            )
                            # causal mask on the diagonal block, in place
                            nc.vector.tensor_mul(
                                pt_c[:, 0:128], pt_c[:, 0:128], tri_sb[:]
                            )
                            pts.append((pt_c, n_c))
                        ptss.append(pts)

                    for hh in range(4):
                        h = 4 * g + hh
                        pts = ptss[hh]
                        # ---- softmax denominator (broadcast to all partitions)
                        r_ps = psG.tile([128, 512], F32, tag="acc")
                        for c in range(4):
                            pt_c, n_c = pts[c]
                            nc.tensor.matmul(
                                r_ps[:, 128 * c:512], ones_sb[:],
                                pt_c[:, 0:n_c],
                                start=(c == 0), stop=(c == 3),
                            )
                        rrec = rrp.tile([128, 512], F32, tag="rr")
                        nc.vector.reciprocal_approx_fast(rrec[:], r_ps[:])

                        # ---- out.T[h] = sum_c V_c.T @ p.T_c, then normalize
                        # (rrec is ready ~0.7us after the R matmuls, so the
                        # mul holds the o_ps PSUM slot only briefly)
                        o_ps = psG.tile([128, 512], F32, tag="acc")
                        for c in range(4):
                            pt_c, n_c = pts[c]
                            nc.tensor.matmul(
                                o_ps[:, 128 * c:512], v_sb[:, c, ts(hh, 128)],
                                pt_c[:, 0:n_c],
                                start=(c == 0), stop=(c == 3),
                            )
                        nc.vector.tensor_mul(
                            attout[:, h, :], o_ps[:], rrec[:]
                        )

                # ---- output projection: y[tok, feat] += bias.
                # One weight fetch per kb chunk feeds all 4 token tiles.
                for nc_ in range(NCH):
                    y_ps = [psG.tile([128, 512], F32, tag="acc",
                                     name=f"y_ps{t}")
                            for t in range(4)]
                    for kb in range(KO // 8):
                        wo_t = wsp.tile([128, 8, 512], BF16, tag="wst")
                        nc.sync.dma_start(
                            wo_t[:], wo[nc_, :, 8 * kb:8 * kb + 8, :]
                        )
                        for r in range(8):
                            ko = 8 * kb + r
                            for t in range(4):
                                nc.tensor.matmul(
                                    y_ps[t][:], attout[:, ko, ts(t, 128)],
                                    wo_t[:, r, :],
                                    start=(ko == 0), stop=(ko == KO - 1),
                                )
                    for t in range(4):
                        y_sb = youtp.tile([128, 512], BF16, tag="y")
                        nc.vector.tensor_add(
                            y_sb[:], y_ps[t][:], bo_sb[:, ts(nc_, 512)]
                        )
                        nc.sync.dma_start(
                            y[512 * b + 128 * t:512 * b + 128 * (t + 1),
                              ts(nc_, 512)],
                            y_sb[:],
                        )

    nc.compile()
    nc.m = get_hw_module(nc.m)
    return nc


def _prep_inputs(x, Wq, bq, Wk, bk, Wv, bv, Wo, bo):
    """Host-side layout prep. Returns the per-core input maps."""
    x = np.asarray(x, dtype=np.float32)
    Wq = np.asarray(Wq, dtype=np.float32)
    Wk = np.asarray(Wk, dtype=np.float32)
    Wv = np.asarray(Wv, dtype=np.float32)
    Wo = np.asarray(Wo, dtype=np.float32)
    bq = np.asarray(bq, dtype=np.float32)
    bk = np.asarray(bk, dtype=np.float32)
    bv = np.asarray(bv, dtype=np.float32)
    bo = np.asarray(bo, dtype=np.float32)

    def lhs_blocks(W):  # [H, 128ki, KO, 128n]: W[128h+n, 128ko+ki]
        return np.ascontiguousarray(
            W.reshape(H, 128, KO, 128).transpose(0, 3, 2, 1)
        ).astype(NPBF16)

    def rhs_blocks(W):  # [NCH, 128ki, KO, 512n]: W[512c+n, 128ko+ki]
        return np.ascontiguousarray(
            W.reshape(NCH, 512, KO, 128).transpose(0, 3, 2, 1)
        ).astype(NPBF16)

    wq_b = lhs_blocks(Wq)
    wk_b = lhs_blocks(Wk)
    wv_b = rhs_blocks(Wv)
    wo_b = rhs_blocks(Wo)
    bqr = np.ascontiguousarray(bq.reshape(H, 128).T)
    bkr = np.ascontiguousarray(bk.reshape(H, 128).T)
    bvb = np.ascontiguousarray(np.broadcast_to(bv, (128, D))).astype(NPBF16)
    bob = np.ascontiguousarray(np.broadcast_to(bo, (128, D))).astype(NPBF16)

    i = np.arange(128)[:, None]
    j = np.arange(128)[None, :]
    tri = (i <= j).astype(NPBF16)
    ones = np.ones((128, 128), dtype=NPBF16)

    in_maps = []
    for core in range(NCORES):
        xc = x[BC * core:BC * (core + 1)]          # [BC, 512, 4096]
        # [BC, 128ki, KO, 512m]: xT[b, ki, ko, m] = x[b, m, 128ko+ki]
        xT = np.ascontiguousarray(
            xc.reshape(BC, 512, KO, 128).transpose(0, 3, 2, 1)
        ).astype(NPBF16)
        in_maps.append({
            "xT": xT, "wq": wq_b, "wk": wk_b, "wv": wv_b, "wo": wo_b,
            "bqr": bqr, "bkr": bkr, "bvb": bvb, "bob": bob,
            "trid": tri, "onesd": ones,
        })
    return in_maps


def _get_nc():
    if "nc" not in _CACHE:
        _CACHE["nc"] = _build()
    return _CACHE["nc"]


def run(trace=False, **inputs):
    """Run on the 8 NeuronCores. Returns (y, BassKernelResults)."""
    nc = _get_nc()
    in_maps = _prep_inputs(**inputs)
    res = bass_utils.run_bass_kernel_spmd(
        nc, in_maps, core_ids=list(range(NCORES)), trace=trace
    )
    y = np.stack(
        [res.results[c]["y"].astype(np.float32) for c in range(NCORES)], axis=0
    )
    y = y.reshape(B, L, D)
    return y, res


def kernel(**inputs):
    y, _ = run(trace=False, **inputs)
    return y


# revision 23
# speedup vs baseline: 1.0195x; 1.0195x over previous
"""Self-contained Trainium2 Bass kernel for causal MHA.

Problem: B=32, L=512, D=4096, H=32 heads (head_dim 128), causal attention,
torch-Linear projections (y = x @ W.T + b).

Strategy: data-parallel over batch across the 8 NeuronCores (4 batches each).
Per core, everything is computed in "transposed activation" layout so no
on-chip transposes are ever needed:
  - Q.T, K.T per head:  [head_dim(part), tok]   (lhsT = W.T tile, rhs = x.T)
  - V natural per head-group: [tok(part), feat]  (lhsT = x.T tile, rhs = Wv.T)
  - S.T = K.T-chunk.T @ Q.T -> [key(part), query]; softmax denominator via an
    all-ones stationary matmul (broadcasts column sums to all partitions);
    causal handled by a multiplicative mask after exp.
  - out.T = V-chunk.T @ p.T accumulated over key chunks -> [head_dim, tok],
    normalized by the reciprocal of the ones-matmul output.
  - y = attout.T-tile.T @ Wo.T chunk (natural layout), + bias, DMA out.
Matmuls run in bf16 (fp32 accumulate in PSUM).

Perf structure:
  - All DRAM tensors pre-arranged on host so every dma_start is a contiguous
    per-partition transfer (one large descriptor per partition).
  - wv/wo weight tiles are loaded once per (batch, chunk) and streamed against
    4 concurrent PSUM accumulation banks (tag "acc", 4 slots) so no weight is
    fetched twice.
  - The softmax denominator PSUM bank is drained by a fast ScalarE copy before
    the slow VectorE reciprocal, so the bank frees in ~0.7us instead of 3.4us
    (which used to stall the next head's matmuls on PSUM slot reuse).
"""

import os
import sys

sys.path.insert(0, "/opt/trn_rl_repo")

import numpy as np
import ml_dtypes

import concourse.bass as bass
import concourse.mybir as mybir
import concourse.tile as tile
from concourse import bacc
from concourse import bass_utils
from concourse.bass_interp import get_hw_module

BF16 = mybir.dt.bfloat16
F32 = mybir.dt.float32
NPBF16 = ml_dtypes.bfloat16
AFT = mybir.ActivationFunctionType

B, L, D, H = 32, 512, 4096, 32
HD = 128
NCORES = 8
BC = B // NCORES          # batches per core
KO = D // 128             # 32 contraction tiles
NCH = D // 512            # 8 output-feature chunks of 512
NG = H // 4               # 8 head groups of 4 heads
SCALE = 1.0 / float(np.sqrt(HD))

_CACHE = {}


def _build():
    nc = bacc.Bacc(
        "TRN2", target_bir_lowering=False, debug=False, enable_asserts=False
    )
    # Pre-transposed on host: per-partition rows are fully contiguous.
    xT = nc.dram_tensor("xT", [BC, 128, KO, 512], BF16, kind="ExternalInput").ap()
    wq = nc.dram_tensor("wq", [H, 128, KO, 128], BF16, kind="ExternalInput").ap()
    wk = nc.dram_tensor("wk", [H, 128, KO, 128], BF16, kind="ExternalInput").ap()
    wv = nc.dram_tensor("wv", [NCH, 128, KO, 512], BF16, kind="ExternalInput").ap()
    wo = nc.dram_tensor("wo", [NCH, 128, KO, 512], BF16, kind="ExternalInput").ap()
    bqr = nc.dram_tensor("bqr", [128, H], F32, kind="ExternalInput").ap()
    bkr = nc.dram_tensor("bkr", [128, H], F32, kind="ExternalInput").ap()
    bvb = nc.dram_tensor("bvb", [128, D], BF16, kind="ExternalInput").ap()
    bob = nc.dram_tensor("bob", [128, D], BF16, kind="ExternalInput").ap()
    trid = nc.dram_tensor("trid", [128, 128], BF16, kind="ExternalInput").ap()
    onesd = nc.dram_tensor("onesd", [128, 128], BF16, kind="ExternalInput").ap()
    y = nc.dram_tensor("y", [BC * 512, D], BF16, kind="ExternalOutput").ap()

    ts = bass.ts

    with tile.TileContext(nc) as tc:
        with tc.tile_pool(name="const", bufs=1) as constp, \
             tc.tile_pool(name="xpool", bufs=1) as xpool, \
             tc.tile_pool(name="wqk", bufs=3) as wqkp, \
             tc.tile_pool(name="wstream", bufs=4) as wsp, \
             tc.tile_pool(name="qk", bufs=5) as qkp, \
             tc.tile_pool(name="vg", bufs=2) as vgp, \
             tc.tile_pool(name="pt", bufs=20) as ptp, \
             tc.tile_pool(name="rr", bufs=2) as rrp, \
             tc.tile_pool(name="att", bufs=1) as attp, \
             tc.tile_pool(name="yout", bufs=4) as youtp, \
             tc.tile_pool(name="psQ", bufs=2, space="PSUM") as psQ, \
             tc.tile_pool(name="psG", bufs=4, space="PSUM") as psG:

            consts_loaded = False

            for b in range(BC):
                xT_sb = xpool.tile([128, KO, 512], BF16, tag="xT")
                # batch 0: stage the x chunks between the weight-stream loads
                # so the very first V matmul has both inputs ASAP
                for kb in ([0] if b == 0 else range(4)):
                    nc.sync.dma_start(
                        xT_sb[:, 8 * kb:8 * kb + 8, :],
                        xT[b, :, 8 * kb:8 * kb + 8, :],
                    )
                attout = attp.tile([128, H, 512], BF16, tag="attout")

                for g in range(NG):
                    # ---- V for the 4 heads of this group: [tok, 512 feats].
                    # One weight fetch per kb chunk feeds all 4 token tiles.
                    v_sb = vgp.tile([128, 4, 512], BF16, tag="vg")
                    v_ps = [psG.tile([128, 512], F32, tag="acc",
                                     name=f"v_ps{t}")
                            for t in range(4)]
                    for kb in range(KO // 8):
                        wv_t = wsp.tile([128, 8, 512], BF16, tag="wst")
                        nc.sync.dma_start(
                            wv_t[:], wv[g, :, 8 * kb:8 * kb + 8, :]
                        )
                        if b == 0 and g == 0 and kb < 3:
                            nc.sync.dma_start(
                                xT_sb[:, 8 * (kb + 1):8 * (kb + 1) + 8, :],
                                xT[b, :, 8 * (kb + 1):8 * (kb + 1) + 8, :],
                            )
                        if not consts_loaded:
                            # issued after the first compute-critical loads so
                            # the first V matmuls start as early as possible
                            consts_loaded = True
                            tri_sb = constp.tile([128, 128], BF16)
                            nc.sync.dma_start(tri_sb[:], trid[:])
                            ones_sb = constp.tile([128, 128], BF16)
                            nc.sync.dma_start(ones_sb[:], onesd[:])
                            bq_sb = constp.tile([128, H], F32)
                            nc.sync.dma_start(bq_sb[:], bqr[:])
                            bk_sb = constp.tile([128, H], F32)
                            nc.sync.dma_start(bk_sb[:], bkr[:])
                            bv_sb = constp.tile([128, D], BF16)
                            nc.sync.dma_start(bv_sb[:], bvb[:])
                            bo_sb = constp.tile([128, D], BF16)
                            nc.sync.dma_start(bo_sb[:], bob[:])
                        for r in range(8):
                            ko = 8 * kb + r
                            for t in range(4):
                                nc.tensor.matmul(
                                    v_ps[t][:], xT_sb[:, ko, ts(t, 128)],
                                    wv_t[:, r, :],
                                    start=(ko == 0), stop=(ko == KO - 1),
                                )
                    for t in range(4):
                        nc.vector.tensor_add(
                            v_sb[:, t, :], v_ps[t][:], bv_sb[:, ts(g, 512)]
                        )

                    # ---- Q.T/K.T for all 4 heads first (gives PE runway to
                    # hide the per-head softmax vector work below)
                    qs, ks = [], []
                    for hh in range(4):
                        h = 4 * g + hh
                        wq_sb = wqkp.tile([128, KO, 128], BF16, tag="w")
                        nc.sync.dma_start(wq_sb[:], wq[h])
                        q_ps = psQ.tile([128, 512], F32, tag="qk")
                        for ko in range(KO):
                            nc.tensor.matmul(
                                q_ps[:], wq_sb[:, ko, :], xT_sb[:, ko, :],
                                start=(ko == 0), stop=(ko == KO - 1),
                            )
                        q_sb = qkp.tile([128, 512], BF16, tag="q")
                        nc.scalar.activation(
                            q_sb[:], q_ps[:], AFT.Identity,
                            bias=bq_sb[:, h:h + 1],
                        )
                        qs.append(q_sb)

                        wk_sb = wqkp.tile([128, KO, 128], BF16, tag="w")
                        nc.sync.dma_start(wk_sb[:], wk[h])
                        k_ps = psQ.tile([128, 512], F32, tag="qk")
                        for ko in range(KO):
                            nc.tensor.matmul(
                                k_ps[:], wk_sb[:, ko, :], xT_sb[:, ko, :],
                                start=(ko == 0), stop=(ko == KO - 1),
                            )
                        k_sb = qkp.tile([128, 512], BF16, tag="k")
                        nc.scalar.activation(
                            k_sb[:], k_ps[:], AFT.Identity,
                            bias=bk_sb[:, h:h + 1],
                        )
                        ks.append(k_sb)

                    # ---- S.T + exp + causal mask for ALL 4 heads first, so
                    # the R/av matmuls of head hh overlap the exp/mask vector
                    # chains of heads hh+1..  Chunk c only contributes to
                    # queries >= 128c (causal): N_c = 512-128c columns.
                    ptss = []
                    for hh in range(4):
                        q_sb, k_sb = qs[hh], ks[hh]
                        pts = []
                        for c in range(4):
                            n_c = 512 - 128 * c
                            st_ps = psQ.tile([128, 512], F32, tag="st")
                            nc.tensor.matmul(
                                st_ps[:, 0:n_c], k_sb[:, ts(c, 128)],
                                q_sb[:, 128 * c:512],
                                start=True, stop=True,
                            )
                            pt_c = ptp.tile([128, 512], BF16, tag="pt")
                            nc.scalar.activation(
                                pt_c[:, 0:n_c], st_ps[:, 0:n_c],
                                AFT.Exp, scale=SCALE,
                            )
                            # causal mask on the diagonal block, in place
                            nc.vector.tensor_mul(
                                pt_c[:, 0:128], pt_c[:, 0:128], tri_sb[:]
                            )
                            pts.append((pt_c, n_c))
                        ptss.append(pts)

                    for hh in range(4):
                        h = 4 * g + hh
                        pts = ptss[hh]
                        # ---- softmax denominator (broadcast to all partitions)
                        r_ps = psG.tile([128, 512], F32, tag="acc")
                        for c in range(4):
                            pt_c, n_c = pts[c]
                            nc.tensor.matmul(
                                r_ps[:, 128 * c:512], ones_sb[:],
                                pt_c[:, 0:n_c],
                                start=(c == 0), stop=(c == 3),
                            )
                        rrec = rrp.tile([128, 512], F32, tag="rr")
                        nc.vector.reciprocal_approx_fast(rrec[:], r_ps[:])

                        # ---- out.T[h] = sum_c V_c.T @ p.T_c, then normalize
                        # (rrec is ready ~0.7us after the R matmuls, so the
                        # mul holds the o_ps PSUM slot only briefly)
                        o_ps = psG.tile([128, 512], F32, tag="acc")
                        for c in range(4):
                            pt_c, n_c = pts[c]
                            nc.tensor.matmul(
                                o_ps[:, 128 * c:512], v_sb[:, c, ts(hh, 128)],
                                pt_c[:, 0:n_c],
                                start=(c == 0), stop=(c == 3),
                            )
                        nc.vector.tensor_mul(
                            attout[:, h, :], o_ps[:], rrec[:]
                        )

                # ---- output projection: y[tok, feat] += bias.
                # One weight fetch per kb chunk feeds all 4 token tiles.
                for nc_ in range(NCH):
                    y_ps = [psG.tile([128, 512], F32, tag="acc",
                                     name=f"y_ps{t}")
                            for t in range(4)]
                    for kb in range(KO // 8):
                        wo_t = wsp.tile([128, 8, 512], BF16, tag="wst")
                        nc.sync.dma_start(
                            wo_t[:], wo[nc_, :, 8 * kb:8 * kb + 8, :]
                        )
                        for r in range(8):
                            ko = 8 * kb + r
                            for t in range(4):
                                nc.tensor.matmul(
                                    y_ps[t][:], attout[:, ko, ts(t, 128)],
                                    wo_t[:, r, :],
                                    start=(ko == 0), stop=(ko == KO - 1),
                                )
                    for t in range(4):
                        y_sb = youtp.tile([128, 512], BF16, tag="y")
                        nc.vector.tensor_add(
                            y_sb[:], y_ps[t][:], bo_sb[:, ts(nc_, 512)]
                        )
                        nc.sync.dma_start(
                            y[512 * b + 128 * t:512 * b + 128 * (t + 1),
                              ts(nc_, 512)],
                            y_sb[:],
                        )

    nc.compile()
    nc.m = get_hw_module(nc.m)
    return nc


def _prep_inputs(x, Wq, bq, Wk, bk, Wv, bv, Wo, bo):
    """Host-side layout prep. Returns the per-core input maps."""
    x = np.asarray(x, dtype=np.float32)
    Wq = np.asarray(Wq, dtype=np.float32)
    Wk = np.asarray(Wk, dtype=np.float32)
    Wv = np.asarray(Wv, dtype=np.float32)
    Wo = np.asarray(Wo, dtype=np.float32)
    bq = np.asarray(bq, dtype=np.float32)
    bk = np.asarray(bk, dtype=np.float32)
    bv = np.asarray(bv, dtype=np.float32)
    bo = np.asarray(bo, dtype=np.float32)

    def lhs_blocks(W):  # [H, 128ki, KO, 128n]: W[128h+n, 128ko+ki]
        return np.ascontiguousarray(
            W.reshape(H, 128, KO, 128).transpose(0, 3, 2, 1)
        ).astype(NPBF16)

    def rhs_blocks(W):  # [NCH, 128ki, KO, 512n]: W[512c+n, 128ko+ki]
        return np.ascontiguousarray(
            W.reshape(NCH, 512, KO, 128).transpose(0, 3, 2, 1)
        ).astype(NPBF16)

    wq_b = lhs_blocks(Wq)
    wk_b = lhs_blocks(Wk)
    wv_b = rhs_blocks(Wv)
    wo_b = rhs_blocks(Wo)
    bqr = np.ascontiguousarray(bq.reshape(H, 128).T)
    bkr = np.ascontiguousarray(bk.reshape(H, 128).T)
    bvb = np.ascontiguousarray(np.broadcast_to(bv, (128, D))).astype(NPBF16)
    bob = np.ascontiguousarray(np.broadcast_to(bo, (128, D))).astype(NPBF16)

    i = np.arange(128)[:, None]
    j = np.arange(128)[None, :]
    tri = (i <= j).astype(NPBF16)
    ones = np.ones((128, 128), dtype=NPBF16)

    in_maps = []
    for core in range(NCORES):
        xc = x[BC * core:BC * (core + 1)]          # [BC, 512, 4096]
        # [BC, 128ki, KO, 512m]: xT[b, ki, ko, m] = x[b, m, 128ko+ki]
        xT = np.ascontiguousarray(
            xc.reshape(BC, 512, KO, 128).transpose(0, 3, 2, 1)
        ).astype(NPBF16)
        in_maps.append({
            "xT": xT, "wq": wq_b, "wk": wk_b, "wv": wv_b, "wo": wo_b,
            "bqr": bqr, "bkr": bkr, "bvb": bvb, "bob": bob,
            "trid": tri, "onesd": ones,
        })
    return in_maps


def _get_nc():
    if "nc" not in _CACHE:
        _CACHE["nc"] = _build()
    return _CACHE["nc"]


def run(trace=False, **inputs):
    """Run on the 8 NeuronCores. Returns (y, BassKernelResults)."""
    nc = _get_nc()
    in_maps = _prep_inputs(**inputs)
    res = bass_utils.run_bass_kernel_spmd(
        nc, in_maps, core_ids=list(range(NCORES)), trace=trace
    )
    y = np.stack(
        [res.results[c]["y"].astype(np.float32) for c in range(NCORES)], axis=0
    )
    y = y.reshape(B, L, D)
    return y, res


def kernel(**inputs):
    y, _ = run(trace=False, **inputs)
    return y
